# revision 30
# baseline (speedup 1.0000x reference)
"""Sliding-window multi-head attention on 8 Trainium2 NeuronCores.

Sharding: tensor-parallel over heads. 16 heads -> 2 heads per core.
Each core computes q/k/v projections for its 2 heads (d' = 128 dims),
banded (window=256) attention for those heads over all tokens, and a
partial output projection (its 128 columns of Wo). Host sums the 8
partials and adds the bias.

Layouts are chosen so every matmul's contraction dim sits on SBUF
partitions with no on-device transposes of activations except v
(32 PE transposes):
  - x is passed pre-transposed from host: xT [8,128,4096]
  - projections produce qT/kT/vT [128 d', 4096 t]
  - scores are computed k-major: sT_j [128 k, 384 q] so softmax's
    denominator comes out of the PV matmul for free (ones column
    appended to v) and no exp-score transposes are needed.
All matmuls use float32r (tf32-like, 1 cycle/row when N>=256).

The attention j-loop is software-pipelined: score j+1 is issued before
the PV matmuls of j so the exp/mask (ACT/DVE) latency of j hides under
PE work, and chunk completions (1/Z replicate + normalize + output
projection) are processed one j late for the same reason.
"""

import sys

sys.path.insert(0, "/opt/trn_rl_repo")

from contextlib import ExitStack

import numpy as np

import concourse.bass as bass
import concourse.tile as tile
from concourse import bacc, mybir
from concourse.bass_utils import run_bass_kernel_spmd

F32 = mybir.dt.float32
F32R = mybir.dt.float32r
ACT_EXP = mybir.ActivationFunctionType.Exp
ACT_COPY = mybir.ActivationFunctionType.Copy

N_CORES = 8
B, S, E = 2, 2048, 1024
H, D = 16, 64
T = B * S                # 4096 tokens total
NB = S // 128            # 16 key/query blocks per batch
PADW = S + 256           # 2304: padded q width per batch
WIN = 384                # q-window per key block (3 blocks)
WOFF = {"wq": 0, "wk": 1024, "wv": 2048, "wo": 3072}


class _Ctx:
    pass


def _emit(tc, io):
    nc = tc.nc
    with ExitStack() as ctx:
        const = ctx.enter_context(tc.tile_pool(name="const", bufs=1))
        big = ctx.enter_context(tc.tile_pool(name="big", bufs=1))
        xpool = ctx.enter_context(tc.tile_pool(name="xload", bufs=3))
        vtmp = ctx.enter_context(tc.tile_pool(name="vtmp", bufs=3))
        expool = ctx.enter_context(tc.tile_pool(name="expool", bufs=10))
        zpool = ctx.enter_context(tc.tile_pool(name="zpool", bufs=6))
        ostage = ctx.enter_context(tc.tile_pool(name="ostage", bufs=6))
        psP = ctx.enter_context(tc.tile_pool(name="psP", bufs=1, space="PSUM"))
        psS = ctx.enter_context(tc.tile_pool(name="psS", bufs=3, space="PSUM"))
        psU = ctx.enter_context(tc.tile_pool(name="psU", bufs=2, space="PSUM"))
        psW = ctx.enter_context(tc.tile_pool(name="psW", bufs=2, space="PSUM"))

        g = _Ctx()

        # ---- constants (packed DMAs; wq first so proj can start asap) --
        wpack = const.tile([128, 4096], F32R, tag="wpack")
        nc.sync.dma_start(wpack[:, 0:1024], io["wpack"][:, 0:1024])
        mpack = const.tile([128, 3 * WIN + 128], F32R, tag="mpack")
        ones = const.tile([1, 128], F32R, tag="ones")
        # ident + ones are needed by the first proj chunk / completions
        nc.sync.dma_start(mpack[:, 3 * WIN : 3 * WIN + 128], io["mpack"][:, 3 * WIN :])
        nc.sync.dma_start(ones[:], io["ones"][:])

        g.w = lambda kind, e: wpack[:, WOFF[kind] + 128 * e : WOFF[kind] + 128 * e + 128]
        g.mask = lambda i: mpack[:, WIN * i : WIN * i + WIN]
        g.ident = mpack[:, 3 * WIN : 3 * WIN + 128]
        g.ones = ones

        def load_rest_weights():
            for kind in ("wk", "wv", "wo"):
                o = WOFF[kind]
                nc.sync.dma_start(wpack[:, o : o + 1024], io["wpack"][:, o : o + 1024])

        def load_small_consts():
            nc.sync.dma_start(mpack[:, 0 : 3 * WIN], io["mpack"][:, 0 : 3 * WIN])

        # ---- persistent activation buffers -----------------------------
        g.qTp = big.tile([128, B * PADW], F32R, tag="qTp")
        g.kT = big.tile([128, T], F32R, tag="kT")
        g.vA = big.tile([128, 32 * 130], F32R, tag="vA")
        g.aoT = big.tile([128, T], F32R, tag="aoT")
        nc.gpsimd.memset(g.qTp[:].bitcast(F32), 0.0)
        nc.gpsimd.memset(g.vA[:].bitcast(F32), 1.0)

        g.wo_ready = []

        def drain_wo(k):
            for _ in range(min(k, len(g.wo_ready))):
                n, m = g.wo_ready.pop(0)
                wps = psW.tile([128, 512], F32, tag="w", name="wps")
                nc.tensor.matmul(
                    wps[:], g.w("wo", m), g.aoT[:, 512 * n : 512 * n + 512],
                    start=True, stop=True,
                )
                ost = ostage.tile([128, 512], F32R, tag="ost", name="ost")
                nc.any.tensor_copy(ost[:], wps[:])
                nc.sync.dma_start(io["outT"][m, :, 512 * n : 512 * n + 512], ost[:])

        # ---- projection t-chunk -----------------------------------------
        def proj_chunk(n):
            xt = xpool.tile([128, 4096], F32R, tag="xt")
            for e in range(8):
                nc.sync.dma_start(
                    xt[:, 512 * e : 512 * e + 512],
                    io["xT"][e, :, 512 * n : 512 * n + 512],
                )
            if getattr(g, "first_chunk_hook", None):
                g.first_chunk_hook()
                g.first_chunk_hook = None
            b, cn = divmod(n, 4)
            for kind in ("wq", "wk", "wv"):
                ps = psP.tile([128, 512], F32, tag="p")
                for e in range(8):
                    nc.tensor.matmul(
                        ps[:], g.w(kind, e), xt[:, 512 * e : 512 * e + 512],
                        start=(e == 0), stop=(e == 7),
                    )
                if kind == "wq":
                    o = PADW * b + 128 + 512 * cn
                    nc.any.tensor_copy(g.qTp[:, o : o + 512], ps[:])
                elif kind == "wk":
                    nc.any.tensor_copy(g.kT[:, 512 * n : 512 * n + 512], ps[:])
                else:
                    vt = vtmp.tile([128, 512], F32R, tag="vt")
                    nc.any.tensor_copy(vt[:], ps[:])
                    for i in range(4):
                        tb = 4 * n + i
                        tps = psP.tile([128, 128], F32R, tag="p")
                        nc.tensor.transpose(tps[:], vt[:, 128 * i : 128 * i + 128], g.ident)
                        nc.any.tensor_copy(g.vA[:, 130 * tb : 130 * tb + 64], tps[:, 0:64])
                        nc.any.tensor_copy(
                            g.vA[:, 130 * tb + 65 : 130 * tb + 129], tps[:, 64:128]
                        )

        # ---- attention stream for one (batch, head) ---------------------
        def attn(b, h):
            def emit_score(j):
                sT = psS.tile([128, WIN], F32, tag="s")
                nc.tensor.matmul(
                    sT[:],
                    g.kT[64 * h : 64 * h + 64, S * b + 128 * j : S * b + 128 * j + 128],
                    g.qTp[64 * h : 64 * h + 64, PADW * b + 128 * j : PADW * b + 128 * j + WIN],
                    start=True, stop=True,
                )
                ex = expool.tile([128, WIN], F32R, tag="ex")
                nc.scalar.activation(ex[:], sT[:], ACT_EXP)
                ex2 = expool.tile([128, WIN], F32R, tag="ex2")
                mi = 0 if j == 0 else (2 if j == NB - 1 else 1)
                nc.vector.tensor_mul(ex2[:], ex[:], g.mask(mi))
                return ex2

            def finish(c, u):
                rz = zpool.tile([1, 512], F32R, tag="rz")
                with nc.allow_low_precision(reason="f32r is fp32-width"):
                    nc.vector.reciprocal(rz[:], u[64:65, :])
                zr = psW.tile([128, 512], F32, tag="w", name="zr")
                nc.tensor.matmul(zr[:], g.ones[:], rz[:], start=True, stop=True)
                dst = g.aoT[64 * h : 64 * h + 64, S * b + 512 * c : S * b + 512 * c + 512]
                nc.scalar.activation(dst, u[0:64, :], ACT_COPY)
                nc.vector.tensor_mul(dst, dst, zr[64 * h : 64 * h + 64, :])
                if h == 1:
                    g.wo_ready.extend((4 * b + c, m) for m in range(8))

            umap, fresh, pend = {}, set(), []
            ex2 = emit_score(0)
            for j in range(NB):
                ex2_next = emit_score(j + 1) if j + 1 < NB else None
                drain_wo(1)
                qlo_w, qhi_w = 128 * (j - 1), 128 * (j + 2)
                tb = NB * b + j
                for c in sorted({max(qlo_w, 0) // 512, (min(qhi_w, S) - 1) // 512}):
                    plo = max(qlo_w, 512 * c, 0)
                    phi = min(qhi_w, 512 * c + 512, S)
                    if plo >= phi:
                        continue
                    if c not in umap:
                        umap[c] = psU.tile([65, 512], F32, tag="u", name="u")
                        fresh.add(c)
                    nc.tensor.matmul(
                        umap[c][:, plo - 512 * c : phi - 512 * c],
                        g.vA[:, 130 * tb + 65 * h : 130 * tb + 65 * h + 65],
                        ex2[:, plo - qlo_w : phi - qlo_w],
                        start=(c in fresh), stop=(j == min(4 * c + 4, NB - 1)),
                        skip_group_check=True,
                    )
                    fresh.discard(c)
                while pend:
                    finish(*pend.pop(0))
                for c in sorted(umap):
                    if j == min(4 * c + 4, NB - 1):
                        pend.append((c, umap.pop(c)))
                ex2 = ex2_next
            while pend:
                finish(*pend.pop(0))

        # ---- schedule: per-batch proj then attention; wo work from a
        # batch drains through the following streams ---------------------
        g.first_chunk_hook = load_rest_weights
        proj_chunk(0)
        load_small_consts()
        for n in range(1, 4):
            proj_chunk(n)
        attn(0, 0)
        attn(0, 1)
        for n in range(4, 8):
            proj_chunk(n)
            drain_wo(4)
        attn(1, 0)
        attn(1, 1)
        drain_wo(64)


def build_program():
    nc = bacc.Bacc("TRN2", target_bir_lowering=False, debug=False, num_devices=N_CORES)
    io = {}

    def inp(name, shape):
        io[name] = nc.dram_tensor(name, shape, F32R, kind="ExternalInput").ap()

    inp("xT", [8, 128, T])
    inp("wpack", [128, 4096])
    inp("mpack", [128, 3 * WIN + 128])
    inp("ones", [1, 128])
    io["outT"] = nc.dram_tensor("outT", [8, 128, T], F32R, kind="ExternalOutput").ap()

    with tile.TileContext(nc) as tc:
        _emit(tc, io)
    nc.compile()
    return nc


def _host_inputs(x, Wq, Wk, Wv, Wo):
    """Per-core input maps (host-side sharding / relayout)."""
    xf = np.ascontiguousarray(x.reshape(T, E).T)            # [1024, 4096]
    xT = xf.reshape(8, 128, T)

    band = np.zeros((128, WIN), dtype=np.float32)
    for r in range(128):
        band[r, r : r + 257] = 1.0                           # |q - k| <= 128
    m_left = band.copy()
    m_left[:, :128] = 0.0
    m_right = band.copy()
    m_right[:, 256:] = 0.0
    mpack = np.concatenate(
        [m_left, band, m_right, np.eye(128, dtype=np.float32)], axis=1
    )
    ones = np.ones((1, 128), dtype=np.float32)

    scale = 1.0 / np.sqrt(D)
    in_maps = []
    for c in range(N_CORES):
        rows = slice(128 * c, 128 * c + 128)
        wq = np.ascontiguousarray((Wq[rows, :] * scale).T)   # [1024 e, 128 d']
        wk = np.ascontiguousarray(Wk[rows, :].T)
        wv = np.ascontiguousarray(Wv[rows, :].T)
        # [8,128,128] lhsT chunks, partition = contraction dim
        wqc = wq.reshape(8, 128, 128)
        wkc = wk.reshape(8, 128, 128)
        wvc = wv.reshape(8, 128, 128)
        woc = Wo[:, rows].T.reshape(128, 8, 128).transpose(1, 0, 2)  # [8,128 d',128 e]
        # pack as [128, 4096]: for chunk e the 128x128 block sits at col 128e
        wpack = np.zeros((128, 4096), dtype=np.float32)
        for e in range(8):
            wpack[:, 0 + 128 * e : 128 * e + 128] = wqc[e]
            wpack[:, 1024 + 128 * e : 1152 + 128 * e] = wkc[e]
            wpack[:, 2048 + 128 * e : 2176 + 128 * e] = wvc[e]
            wpack[:, 3072 + 128 * e : 3200 + 128 * e] = woc[e]
        in_maps.append({"xT": xT, "wpack": wpack, "mpack": mpack, "ones": ones})
    return in_maps


_NC_CACHE = None


def kernel(x, Wq, Wk, Wv, Wo, bo):
    global _NC_CACHE
    x = np.asarray(x, dtype=np.float32)
    Wq = np.asarray(Wq, dtype=np.float32)
    Wk = np.asarray(Wk, dtype=np.float32)
    Wv = np.asarray(Wv, dtype=np.float32)
    Wo = np.asarray(Wo, dtype=np.float32)
    bo = np.asarray(bo, dtype=np.float32)

    if _NC_CACHE is None:
        _NC_CACHE = build_program()
    nc = _NC_CACHE

    in_maps = _host_inputs(x, Wq, Wk, Wv, Wo)
    res = run_bass_kernel_spmd(nc, in_maps, core_ids=list(range(N_CORES)))

    acc = np.zeros((E, T), dtype=np.float32)
    for c in range(N_CORES):
        acc += res.results[c]["outT"].reshape(E, T)
    out = acc.T + bo[None, :]
    return np.ascontiguousarray(out.reshape(B, S, E))


# revision 31
# speedup vs baseline: 1.1601x; 1.1601x over previous
"""Sliding-window multi-head attention on 8 Trainium2 NeuronCores.

Sharding: tensor-parallel over heads. 16 heads -> 2 heads per core.
Each core computes q/k/v projections for its 2 heads (d' = 128 dims),
banded (window=256) attention for those heads over all tokens, and a
partial output projection (its 128 columns of Wo). Host sums the 8
partials and adds the bias.

Layouts are chosen so every matmul's contraction dim sits on SBUF
partitions with no on-device transposes of activations except v
(32 PE transposes):
  - x is passed pre-transposed from host: xT [8,128,4096]
  - projections produce qT/kT/vT [128 d', 4096 t]
  - scores are computed k-major: sT_j [128 k, 384 q] so softmax's
    denominator comes out of the PV matmul for free (ones column
    appended to v) and no exp-score transposes are needed.
All matmuls use float32r (tf32-like, 1 cycle/row when N>=256).

The attention j-loop is software-pipelined: score j+1 is issued before
the PV matmuls of j so the exp/mask (ACT/DVE) latency of j hides under
PE work, and chunk completions (1/Z replicate + normalize + output
projection) are processed one j late for the same reason.
"""

import sys

sys.path.insert(0, "/opt/trn_rl_repo")

from contextlib import ExitStack

import numpy as np

import concourse.bass as bass
import concourse.tile as tile
from concourse import bacc, mybir
from concourse.bass_utils import run_bass_kernel_spmd

F32 = mybir.dt.float32
F32R = mybir.dt.float32r
ACT_EXP = mybir.ActivationFunctionType.Exp
ACT_COPY = mybir.ActivationFunctionType.Copy

N_CORES = 8
B, S, E = 2, 2048, 1024
H, D = 16, 64
T = B * S                # 4096 tokens total
NB = S // 128            # 16 key/query blocks per batch
PADW = S + 256           # 2304: padded q width per batch
WIN = 384                # q-window per key block (3 blocks)
WOFF = {"wq": 0, "wk": 1024, "wv": 2048, "wo": 3072}


class _Ctx:
    pass


def _emit(tc, io):
    nc = tc.nc
    with ExitStack() as ctx:
        const = ctx.enter_context(tc.tile_pool(name="const", bufs=1))
        big = ctx.enter_context(tc.tile_pool(name="big", bufs=1))
        xpool = ctx.enter_context(tc.tile_pool(name="xload", bufs=3))
        vtmp = ctx.enter_context(tc.tile_pool(name="vtmp", bufs=3))
        expool = ctx.enter_context(tc.tile_pool(name="expool", bufs=10))
        zpool = ctx.enter_context(tc.tile_pool(name="zpool", bufs=6))
        ostage = ctx.enter_context(tc.tile_pool(name="ostage", bufs=6))
        psP = ctx.enter_context(tc.tile_pool(name="psP", bufs=2, space="PSUM"))
        psS = ctx.enter_context(tc.tile_pool(name="psS", bufs=2, space="PSUM"))
        psU = ctx.enter_context(tc.tile_pool(name="psU", bufs=2, space="PSUM"))
        psW = ctx.enter_context(tc.tile_pool(name="psW", bufs=2, space="PSUM"))

        g = _Ctx()

        # ---- constants (packed DMAs; wq first so proj can start asap) --
        wpack = const.tile([128, 4096], F32R, tag="wpack")
        nc.sync.dma_start(wpack[:, 0:1024], io["wpack"][:, 0:1024])
        mpack = const.tile([128, 3 * WIN + 128], F32R, tag="mpack")
        ones = const.tile([1, 128], F32R, tag="ones")
        # ident + ones are needed by the first proj chunk / completions
        nc.sync.dma_start(mpack[:, 3 * WIN : 3 * WIN + 128], io["mpack"][:, 3 * WIN :])
        nc.sync.dma_start(ones[:], io["ones"][:])

        g.w = lambda kind, e: wpack[:, WOFF[kind] + 128 * e : WOFF[kind] + 128 * e + 128]
        g.mask = lambda i: mpack[:, WIN * i : WIN * i + WIN]
        g.ident = mpack[:, 3 * WIN : 3 * WIN + 128]
        g.ones = ones

        def load_rest_weights():
            for kind in ("wk", "wv", "wo"):
                o = WOFF[kind]
                nc.sync.dma_start(wpack[:, o : o + 1024], io["wpack"][:, o : o + 1024])

        def load_small_consts():
            nc.sync.dma_start(mpack[:, 0 : 3 * WIN], io["mpack"][:, 0 : 3 * WIN])

        # ---- persistent activation buffers -----------------------------
        g.qTp = big.tile([128, B * PADW], F32R, tag="qTp")
        g.kT = big.tile([128, T], F32R, tag="kT")
        g.vA = big.tile([128, 32 * 130], F32R, tag="vA")
        g.aoT = big.tile([128, T], F32R, tag="aoT")
        nc.gpsimd.memset(g.qTp[:].bitcast(F32), 0.0)
        nc.gpsimd.memset(g.vA[:].bitcast(F32), 1.0)

        g.wo_ready = []

        def drain_wo(k):
            for _ in range(min(k, len(g.wo_ready))):
                n, m = g.wo_ready.pop(0)
                wps = psW.tile([128, 512], F32, tag="w", name="wps")
                nc.tensor.matmul(
                    wps[:], g.w("wo", m), g.aoT[:, 512 * n : 512 * n + 512],
                    start=True, stop=True,
                )
                ost = ostage.tile([128, 512], F32R, tag="ost", name="ost")
                nc.any.tensor_copy(ost[:], wps[:])
                nc.sync.dma_start(io["outT"][m, :, 512 * n : 512 * n + 512], ost[:])

        # ---- projection t-chunk -----------------------------------------
        def proj_chunk(n):
            xt = xpool.tile([128, 4096], F32R, tag="xt")
            for e in range(8):
                nc.sync.dma_start(
                    xt[:, 512 * e : 512 * e + 512],
                    io["xT"][e, :, 512 * n : 512 * n + 512],
                )
            if getattr(g, "first_chunk_hook", None):
                g.first_chunk_hook()
                g.first_chunk_hook = None
            b, cn = divmod(n, 4)
            for kind in ("wq", "wk", "wv"):
                ps = psP.tile([128, 512], F32, tag="p")
                for e in range(8):
                    nc.tensor.matmul(
                        ps[:], g.w(kind, e), xt[:, 512 * e : 512 * e + 512],
                        start=(e == 0), stop=(e == 7),
                    )
                if kind == "wq":
                    o = PADW * b + 128 + 512 * cn
                    nc.any.tensor_copy(g.qTp[:, o : o + 512], ps[:])
                elif kind == "wk":
                    nc.any.tensor_copy(g.kT[:, 512 * n : 512 * n + 512], ps[:])
                else:
                    vt = vtmp.tile([128, 512], F32R, tag="vt")
                    nc.any.tensor_copy(vt[:], ps[:])
                    for i in range(4):
                        tb = 4 * n + i
                        tps = psP.tile([128, 128], F32R, tag="p")
                        nc.tensor.transpose(tps[:], vt[:, 128 * i : 128 * i + 128], g.ident)
                        nc.any.tensor_copy(g.vA[:, 130 * tb : 130 * tb + 64], tps[:, 0:64])
                        nc.any.tensor_copy(
                            g.vA[:, 130 * tb + 65 : 130 * tb + 129], tps[:, 64:128]
                        )

        # ---- attention stream for one (batch, head) ---------------------
        def attn(b, h):
            def emit_score(j):
                sT = psS.tile([128, WIN], F32, tag="s")
                nc.tensor.matmul(
                    sT[:],
                    g.kT[64 * h : 64 * h + 64, S * b + 128 * j : S * b + 128 * j + 128],
                    g.qTp[64 * h : 64 * h + 64, PADW * b + 128 * j : PADW * b + 128 * j + WIN],
                    start=True, stop=True,
                )
                ex = expool.tile([128, WIN], F32R, tag="ex")
                nc.scalar.activation(ex[:], sT[:], ACT_EXP)
                ex2 = expool.tile([128, WIN], F32R, tag="ex2")
                mi = 0 if j == 0 else (2 if j == NB - 1 else 1)
                nc.vector.tensor_mul(ex2[:], ex[:], g.mask(mi))
                return ex2

            def finish(c, u):
                rz = zpool.tile([1, 512], F32R, tag="rz")
                with nc.allow_low_precision(reason="f32r is fp32-width"):
                    nc.vector.reciprocal(rz[:], u[64:65, :])
                zr = psW.tile([128, 512], F32, tag="w", name="zr")
                nc.tensor.matmul(zr[:], g.ones[:], rz[:], start=True, stop=True)
                dst = g.aoT[64 * h : 64 * h + 64, S * b + 512 * c : S * b + 512 * c + 512]
                nc.scalar.activation(dst, u[0:64, :], ACT_COPY)
                nc.vector.tensor_mul(dst, dst, zr[64 * h : 64 * h + 64, :])
                if h == 1:
                    g.wo_ready.extend((4 * b + c, m) for m in range(8))

            umap, fresh, pend = {}, set(), []
            ex2 = emit_score(0)
            for j in range(NB):
                ex2_next = emit_score(j + 1) if j + 1 < NB else None
                drain_wo(1)
                qlo_w, qhi_w = 128 * (j - 1), 128 * (j + 2)
                tb = NB * b + j
                for c in sorted({max(qlo_w, 0) // 512, (min(qhi_w, S) - 1) // 512}):
                    plo = max(qlo_w, 512 * c, 0)
                    phi = min(qhi_w, 512 * c + 512, S)
                    if plo >= phi:
                        continue
                    if c not in umap:
                        umap[c] = psU.tile([65, 512], F32, tag="u", name="u")
                        fresh.add(c)
                    nc.tensor.matmul(
                        umap[c][:, plo - 512 * c : phi - 512 * c],
                        g.vA[:, 130 * tb + 65 * h : 130 * tb + 65 * h + 65],
                        ex2[:, plo - qlo_w : phi - qlo_w],
                        start=(c in fresh), stop=(j == min(4 * c + 4, NB - 1)),
                        skip_group_check=True,
                    )
                    fresh.discard(c)
                while pend:
                    finish(*pend.pop(0))
                for c in sorted(umap):
                    if j == min(4 * c + 4, NB - 1):
                        pend.append((c, umap.pop(c)))
                ex2 = ex2_next
            while pend:
                finish(*pend.pop(0))

        # ---- schedule: per-batch proj then attention; wo work from a
        # batch drains through the following streams ---------------------
        g.first_chunk_hook = load_rest_weights
        proj_chunk(0)
        load_small_consts()
        for n in range(1, 4):
            proj_chunk(n)
        attn(0, 0)
        attn(0, 1)
        for n in range(4, 8):
            proj_chunk(n)
            drain_wo(4)
        attn(1, 0)
        attn(1, 1)
        drain_wo(64)


def build_program():
    nc = bacc.Bacc("TRN2", target_bir_lowering=False, debug=False, num_devices=N_CORES)
    io = {}

    def inp(name, shape):
        io[name] = nc.dram_tensor(name, shape, F32R, kind="ExternalInput").ap()

    inp("xT", [8, 128, T])
    inp("wpack", [128, 4096])
    inp("mpack", [128, 3 * WIN + 128])
    inp("ones", [1, 128])
    io["outT"] = nc.dram_tensor("outT", [8, 128, T], F32R, kind="ExternalOutput").ap()

    with tile.TileContext(nc) as tc:
        _emit(tc, io)
    nc.compile()
    return nc


def _host_inputs(x, Wq, Wk, Wv, Wo):
    """Per-core input maps (host-side sharding / relayout)."""
    xf = np.ascontiguousarray(x.reshape(T, E).T)            # [1024, 4096]
    xT = xf.reshape(8, 128, T)

    band = np.zeros((128, WIN), dtype=np.float32)
    for r in range(128):
        band[r, r : r + 257] = 1.0                           # |q - k| <= 128
    m_left = band.copy()
    m_left[:, :128] = 0.0
    m_right = band.copy()
    m_right[:, 256:] = 0.0
    mpack = np.concatenate(
        [m_left, band, m_right, np.eye(128, dtype=np.float32)], axis=1
    )
    ones = np.ones((1, 128), dtype=np.float32)

    scale = 1.0 / np.sqrt(D)
    in_maps = []
    for c in range(N_CORES):
        rows = slice(128 * c, 128 * c + 128)
        wq = np.ascontiguousarray((Wq[rows, :] * scale).T)   # [1024 e, 128 d']
        wk = np.ascontiguousarray(Wk[rows, :].T)
        wv = np.ascontiguousarray(Wv[rows, :].T)
        # [8,128,128] lhsT chunks, partition = contraction dim
        wqc = wq.reshape(8, 128, 128)
        wkc = wk.reshape(8, 128, 128)
        wvc = wv.reshape(8, 128, 128)
        woc = Wo[:, rows].T.reshape(128, 8, 128).transpose(1, 0, 2)  # [8,128 d',128 e]
        # pack as [128, 4096]: for chunk e the 128x128 block sits at col 128e
        wpack = np.zeros((128, 4096), dtype=np.float32)
        for e in range(8):
            wpack[:, 0 + 128 * e : 128 * e + 128] = wqc[e]
            wpack[:, 1024 + 128 * e : 1152 + 128 * e] = wkc[e]
            wpack[:, 2048 + 128 * e : 2176 + 128 * e] = wvc[e]
            wpack[:, 3072 + 128 * e : 3200 + 128 * e] = woc[e]
        in_maps.append({"xT": xT, "wpack": wpack, "mpack": mpack, "ones": ones})
    return in_maps


_NC_CACHE = None


def kernel(x, Wq, Wk, Wv, Wo, bo):
    global _NC_CACHE
    x = np.asarray(x, dtype=np.float32)
    Wq = np.asarray(Wq, dtype=np.float32)
    Wk = np.asarray(Wk, dtype=np.float32)
    Wv = np.asarray(Wv, dtype=np.float32)
    Wo = np.asarray(Wo, dtype=np.float32)
    bo = np.asarray(bo, dtype=np.float32)

    if _NC_CACHE is None:
        _NC_CACHE = build_program()
    nc = _NC_CACHE

    in_maps = _host_inputs(x, Wq, Wk, Wv, Wo)
    res = run_bass_kernel_spmd(nc, in_maps, core_ids=list(range(N_CORES)))

    acc = np.zeros((E, T), dtype=np.float32)
    for c in range(N_CORES):
        acc += res.results[c]["outT"].reshape(E, T)
    out = acc.T + bo[None, :]
    return np.ascontiguousarray(out.reshape(B, S, E))
